# revision 37
# baseline (speedup 1.0000x reference)
"""DCT-II enhancement kernel for Trainium2 (8 NeuronCores, data parallel).

Computes out[b, n, k] = sum_d x[b, n, d] * C[k, d] where C is the 256x256
orthonormal DCT-II basis — i.e. a [B*N, 256] @ [256, 256]^T GEMM.

Sharding: pure data parallel over the flattened token dim (B*N = 131072),
16384 tokens per core. The DCT basis is replicated.

Precision: the correctness gate is rel_err < 2e-2; the orthonormal basis
preserves norms, so bf16 I/O contributes only ~3e-3 relative error while
halving HBM traffic vs fp32.

Symmetry: C[k, 255-d] = (-1)^k C[k, d], so with e[t,d'] = x[t,d']+x[t,255-d']
and o[t,d'] = x[t,d']-x[t,255-d'] (d' < 128):
  out[t, 2k'] = e[t] . C[2k', :128],  out[t, 2k'+1] = o[t] . C[2k'+1, :128]
This halves the PE work (contraction 128 instead of 256) at identical HBM
bytes. The host uploads xt = [e^T; o^T] [256, 16384] bf16 per core; the
device keeps Ce^T / Co^T STATIONARY (one 128x128 weight tile per parity)
and streams tokens as the moving operand, producing a TRANSPOSED output
outT [256(k-packed), 16384(t)] bf16 that the host de-interleaves:
out[t, 2k'+c] = outT[c*128+k', t].

Ring/queue facts measured from traces: per-core HBM sustains ~410 GB/s
aggregate; three DMA rings exist (HWDGE on sync/scalar, SWDGE on gpsimd,
~190-320 GB/s each when fed); 4KB descriptors everywhere.

  per 1024-token iteration:
    1. DMA in xt tile [128p(d'), 2c(e/o), tok] bf16 (4KB runs, 2048-token
       granules, mostly gpsimd SWDGE, sync early).
    2. 4 matmuls (2 per parity, 512 moving tokens each, single-shot
       accumulation) into 2 two-bank PSUM tiles: outT[k' 128, tok 1024].
    3. 2 PSUM->SBUF copies with fp32->bf16 cast (even on DVE, odd on ACT)
       into per-2-iteration [128, 2048] staging tiles.
    4. DMA out even rows (scalar ring) / odd rows (sync after its input
       is done) as [128, 2048] = 4KB contiguous runs; final iterations
       ship per-iteration halves for a fast tail drain.

Measured journey (8 cores live): 102.8us fp32 -> 61.7us bf16 natural ->
this layout. Fixed costs every NEFF pays: ~6us init (outside the
measured window), ~3us DGE spin-up, ~9us epilogue (250-semaphore sweep
+ final barrier).
"""

from contextlib import ExitStack

import ml_dtypes
import numpy as np

import concourse.bass as bass
import concourse.tile as tile
from concourse import bacc, mybir
from concourse.bass_utils import run_bass_kernel_spmd

P = 128
D = 256
N_CORES = 8
B, N = 32, 4096
TOK_PER_CORE = (B * N) // N_CORES  # 16384

F32 = mybir.dt.float32
BF16 = mybir.dt.bfloat16
NP_BF16 = ml_dtypes.bfloat16


def dct_matrix() -> np.ndarray:
    """C[k, d] — DCT-II with ortho normalization, fp64 math cast to fp32."""
    n = D
    k = np.arange(n)[:, None].astype(np.float64)
    m = np.arange(n)[None, :].astype(np.float64)
    Cm = np.cos(np.pi * (2.0 * m + 1.0) * k / (2.0 * n))
    scale = np.full((n, 1), np.sqrt(2.0 / n))
    scale[0, 0] = np.sqrt(1.0 / n)
    return (Cm * scale).astype(np.float32)


def build_program(tok: int = TOK_PER_CORE, super_tok: int = 1024,
                  num_devices: int = N_CORES) -> bass.Bass:
    """Emit the per-core Bass/Tile program. All cores run the same NEFF."""
    assert tok % super_tok == 0
    nit = tok // super_tok       # 1024-token iterations (16)
    gr = 2 * super_tok           # 2048-token granules (input and output)
    ngr = tok // gr
    half_ps = super_tok // 2     # 512 moving tokens per matmul

    nc = bacc.Bacc(
        "TRN2", target_bir_lowering=False, debug=False, num_devices=num_devices
    )
    # rows 0-127: e^T (d'), rows 128-255: o^T
    xt_d = nc.dram_tensor("xt", [D, tok], BF16, kind="ExternalInput").ap()
    # [d', {Ce^T | Co^T}] packed: ct[d', c*128 + k'] = C[2k'+c, d']
    ct_d = nc.dram_tensor("ct", [P, D], BF16, kind="ExternalInput").ap()
    # rows 0-127: even k' outputs, rows 128-255: odd
    out_d = nc.dram_tensor("out", [D, tok], BF16, kind="ExternalOutput").ap()

    with ExitStack() as ctx:
        tc = ctx.enter_context(tile.TileContext(nc))
        consts = ctx.enter_context(tc.tile_pool(name="consts", bufs=1))
        fill_pool = ctx.enter_context(tc.tile_pool(name="xfill", bufs=1))
        xin_pool = ctx.enter_context(tc.tile_pool(name="xin", bufs=5))
        out_sb_pool = ctx.enter_context(tc.tile_pool(name="out_sb", bufs=4))
        # Each PSUM tile spans 2 banks ([128, 1024] fp32); 4 bufs = all 8
        # banks, 2 iterations in flight.
        out_ps_pool = ctx.enter_context(
            tc.tile_pool(name="out_ps", bufs=4, space="PSUM")
        )

        # Stationary weights: 64KB, first on the scalar ring.
        ct_sb = consts.tile([P, 2, P], BF16)
        nc.scalar.dma_start(ct_sb[:], ct_d.rearrange("p (c k) -> p c k", k=P))

        x_q = xt_d.rearrange("(c p) (q t) -> q p c t", p=P, t=super_tok // 4)
        x_half = xt_d.rearrange("(c p) (h t) -> h p c t", p=P, t=super_tok // 2)
        x_fill = xt_d.rearrange("(c p) (i t) -> i p c t", p=P, t=super_tok)
        x_gr = xt_d.rearrange("(c p) (g t) -> g p c t", p=P, t=gr)

        xins = {}

        def stage_a_fill0():
            """Iteration 0 lands as 256/256/512-token tiles so the first
            matmuls start after only 128KB of input."""
            qa = fill_pool.tile([P, 2, super_tok // 4], BF16, name="xf0a")
            qb = fill_pool.tile([P, 2, super_tok // 4], BF16, name="xf0b")
            hc = fill_pool.tile([P, 2, super_tok // 2], BF16, name="xf0c")
            nc.sync.dma_start(qa[:], x_q[0])
            nc.sync.dma_start(qb[:], x_q[1])
            nc.sync.dma_start(hc[:], x_half[1])
            q = super_tok // 4
            # segments: (tile, tile_col0, iter_col0, width)
            xins[0] = [(qa, 0, 0, q), (qb, 0, q, q), (hc, 0, 2 * q, 2 * q)]

        def stage_a_fill1():
            xc = fill_pool.tile([P, 2, super_tok], BF16, name="xfill1")
            nc.gpsimd.dma_start(xc[:], x_fill[1])
            xins[1] = [(xc, 0, 0, super_tok)]

        # Granule rings: sync takes g2 (its fill work ends early), gpsimd
        # (SWDGE) the rest; sync then mostly ships odd-row outputs.
        GR_SYNC = {2}

        def stage_a(g):
            """Granule g covers iterations 2g, 2g+1 (g >= 1). The last
            granule lands as two single-iteration tiles (same gpsimd queue
            position) so iteration 14's compute overlaps iteration 15's
            input transfer."""
            if not (1 <= g < ngr):
                return
            if g == ngr - 1:
                for i in (2 * g, 2 * g + 1):
                    xc = xin_pool.tile([P, 2, super_tok], BF16)
                    nc.gpsimd.dma_start(xc[:], x_fill[i])
                    xins[i] = [(xc, 0, 0, super_tok)]
                return
            xg = xin_pool.tile([P, 2, gr], BF16)
            eng = nc.sync if g in GR_SYNC else nc.gpsimd
            eng.dma_start(xg[:], x_gr[g])
            xins[2 * g] = [(xg, 0, 0, super_tok)]
            xins[2 * g + 1] = [(xg, super_tok, 0, super_tok)]

        pss_by_iter = {}

        def stage_b(i):
            """Per parity: one 2-bank PSUM tile [k' 128, tok 1024], filled
            by single-shot matmuls (contraction 128 = one weight tile),
            moving chunks <= 512 so no chunk straddles a PSUM bank."""
            if not (0 <= i < nit):
                return
            segs = xins.pop(i)
            pss = []
            for par in range(2):
                ps = out_ps_pool.tile([P, super_tok], F32)
                pss.append(ps)
                for (t, tcol0, icol0, width) in segs:
                    for w0 in range(0, width, half_ps):
                        w = min(half_ps, width - w0)
                        nc.tensor.matmul(
                            ps[:, icol0 + w0:icol0 + w0 + w],
                            ct_sb[:, par, :],
                            t[:, par, tcol0 + w0:tcol0 + w0 + w],
                            start=True,
                            stop=True,
                        )
            pss_by_iter[i] = pss

        out_sbs = {}

        # Odd-row out DMAs ride sync from granule 1 on; granule 0's odd
        # rows go on scalar (sync is still landing its input then).
        def out_rings(g):
            ring_e = nc.scalar
            ring_o = nc.scalar if g == 0 else nc.sync
            return ring_e, ring_o

        def stage_c(i):
            """PSUM->SBUF bf16 copies (even->DVE, odd->ACT) into 2-iter
            staging tiles; ship [128, 2048] per parity per granule."""
            if not (0 <= i < nit):
                return
            pss = pss_by_iter.pop(i)
            g, h = divmod(i, 2)
            if h == 0:
                sbe = out_sb_pool.tile([P, gr], BF16, name="sbe")
                sbo = out_sb_pool.tile([P, gr], BF16, name="sbo")
                out_sbs[g] = (sbe, sbo)
            sbe, sbo = out_sbs[g]
            sl = slice(h * super_tok, (h + 1) * super_tok)
            cols = slice(g * gr, (g + 1) * gr)
            ring_e, ring_o = out_rings(g)
            if i >= nit - 2:
                # Tail taper: copy and ship 512-token quarters as they
                # exist, so the chain after the last input byte is one
                # quarter deep instead of a whole iteration.
                for qh in range(2):
                    qsl = slice(sl.start + qh * half_ps,
                                sl.start + (qh + 1) * half_ps)
                    qcols = slice(i * super_tok + qh * half_ps,
                                  i * super_tok + (qh + 1) * half_ps)
                    psl = slice(qh * half_ps, (qh + 1) * half_ps)
                    nc.vector.tensor_copy(sbe[:, qsl], pss[0][:, psl])
                    ring_e.dma_start(out_d[0:P, qcols], sbe[:, qsl])
                    nc.scalar.copy(sbo[:, qsl], pss[1][:, psl])
                    ring_o.dma_start(out_d[P:D, qcols], sbo[:, qsl])
                if h == 1:
                    out_sbs.pop(g)
            else:
                nc.vector.tensor_copy(sbe[:, sl], pss[0][:])
                nc.scalar.copy(sbo[:, sl], pss[1][:])
                if h == 1:
                    ring_e.dma_start(out_d[0:P, cols], sbe[:])
                    ring_o.dma_start(out_d[P:D, cols], sbo[:])
                    out_sbs.pop(g)

        stage_a_fill0()
        stage_a_fill1()
        stage_a(1)
        for i in range(nit + 1):
            if i % 2 == 0:
                stage_a(i // 2 + 2)
            stage_b(i)
            stage_c(i - 1)

    nc.compile()
    return nc


_PROGRAM_CACHE: dict = {}


def _get_program() -> bass.Bass:
    if "nc" not in _PROGRAM_CACHE:
        _PROGRAM_CACHE["nc"] = build_program()
    return _PROGRAM_CACHE["nc"]


def make_in_maps(x_flat: np.ndarray) -> list[dict]:
    """x_flat: [B*N, D] float32. Per core upload xt = [e^T; o^T] bf16 and
    the packed stationary weights ct[d', c*128+k'] = C[2k'+c, d']."""
    C = dct_matrix().astype(np.float64)
    ce = C[0::2, 0:P].T  # [d', k'] even
    co = C[1::2, 0:P].T  # [d', k'] odd
    ct = np.concatenate([ce, co], axis=1).astype(NP_BF16)  # [128, 256]
    ct = np.ascontiguousarray(ct)

    xs = x_flat.reshape(N_CORES, TOK_PER_CORE, D)
    a = xs[:, :, 0:P].astype(np.float32)
    b = xs[:, :, :P - 1:-1].astype(np.float32)  # cols 255..128 = x[255-d']
    e = (a + b).astype(NP_BF16)
    o = (a - b).astype(NP_BF16)
    # [core, t, d'] -> [core, d', t]; stack e over o -> [core, 256, T]
    xt = np.concatenate(
        [e.transpose(0, 2, 1), o.transpose(0, 2, 1)], axis=1
    )
    xt = np.ascontiguousarray(xt)
    return [{"xt": xt[i], "ct": ct} for i in range(N_CORES)]


def kernel(x: np.ndarray) -> np.ndarray:
    x = np.ascontiguousarray(np.asarray(x, dtype=np.float32))
    b, n, d = x.shape
    assert (b, n, d) == (B, N, D), f"unexpected shape {x.shape}"
    nc = _get_program()
    in_maps = make_in_maps(x.reshape(b * n, d))
    res = run_bass_kernel_spmd(nc, in_maps, core_ids=list(range(N_CORES)))
    outs = []
    for r in res.results:
        od = np.asarray(r["out"]).astype(np.float32)  # [256, T] k-packed
        # out[t, 2k'+c] = od[c*128+k', t]
        outs.append(od.reshape(2, P, TOK_PER_CORE).transpose(2, 1, 0)
                    .reshape(TOK_PER_CORE, D))
    return np.concatenate(outs, axis=0).reshape(b, n, d)


# revision 38
# speedup vs baseline: 1.1231x; 1.1231x over previous
"""DCT-II enhancement kernel for Trainium2 (8 NeuronCores, data parallel).

Computes out[b, n, k] = sum_d x[b, n, d] * C[k, d] where C is the 256x256
orthonormal DCT-II basis — i.e. a [B*N, 256] @ [256, 256]^T GEMM.

Sharding: pure data parallel over the flattened token dim (B*N = 131072),
16384 tokens per core. The DCT basis is replicated.

Precision: the correctness gate is rel_err < 2e-2; the orthonormal basis
preserves norms, so bf16 I/O contributes only ~3e-3 relative error while
halving HBM traffic vs fp32.

Symmetry: C[k, 255-d] = (-1)^k C[k, d], so with e[t,d'] = x[t,d']+x[t,255-d']
and o[t,d'] = x[t,d']-x[t,255-d'] (d' < 128):
  out[t, 2k'] = e[t] . C[2k', :128],  out[t, 2k'+1] = o[t] . C[2k'+1, :128]
This halves the PE work (contraction 128 instead of 256) at identical HBM
bytes. The host uploads xt = [e^T; o^T] [256, 16384] bf16 per core; the
device keeps Ce^T / Co^T STATIONARY (one 128x128 weight tile per parity)
and streams tokens as the moving operand, producing a TRANSPOSED output
outT [256(k-packed), 16384(t)] bf16 that the host de-interleaves:
out[t, 2k'+c] = outT[c*128+k', t].

Ring/queue facts measured from traces: per-core HBM sustains ~410 GB/s
aggregate; three DMA rings exist (HWDGE on sync/scalar, SWDGE on gpsimd,
~190-320 GB/s each when fed); 4KB descriptors everywhere.

  per 1024-token iteration:
    1. DMA in xt tile [128p(d'), 2c(e/o), tok] bf16 (4KB runs, 2048-token
       granules, mostly gpsimd SWDGE, sync early).
    2. 4 matmuls (2 per parity, 512 moving tokens each, single-shot
       accumulation) into 2 two-bank PSUM tiles: outT[k' 128, tok 1024].
    3. 2 PSUM->SBUF copies with fp32->bf16 cast (even on DVE, odd on ACT)
       into per-2-iteration [128, 2048] staging tiles.
    4. DMA out even rows (scalar ring) / odd rows (sync after its input
       is done) as [128, 2048] = 4KB contiguous runs; final iterations
       ship per-iteration halves for a fast tail drain.

Measured journey (8 cores live): 102.8us fp32 -> 61.7us bf16 natural ->
this layout. Fixed costs every NEFF pays: ~6us init (outside the
measured window), ~3us DGE spin-up, ~9us epilogue (250-semaphore sweep
+ final barrier).
"""

from contextlib import ExitStack

import ml_dtypes
import numpy as np

import concourse.bass as bass
import concourse.tile as tile
from concourse import bacc, mybir
from concourse.bass_utils import run_bass_kernel_spmd

P = 128
D = 256
N_CORES = 8
B, N = 32, 4096
TOK_PER_CORE = (B * N) // N_CORES  # 16384

F32 = mybir.dt.float32
BF16 = mybir.dt.bfloat16
NP_BF16 = ml_dtypes.bfloat16


def dct_matrix() -> np.ndarray:
    """C[k, d] — DCT-II with ortho normalization, fp64 math cast to fp32."""
    n = D
    k = np.arange(n)[:, None].astype(np.float64)
    m = np.arange(n)[None, :].astype(np.float64)
    Cm = np.cos(np.pi * (2.0 * m + 1.0) * k / (2.0 * n))
    scale = np.full((n, 1), np.sqrt(2.0 / n))
    scale[0, 0] = np.sqrt(1.0 / n)
    return (Cm * scale).astype(np.float32)


def build_program(tok: int = TOK_PER_CORE, super_tok: int = 1024,
                  num_devices: int = N_CORES) -> bass.Bass:
    """Emit the per-core Bass/Tile program. All cores run the same NEFF."""
    assert tok % super_tok == 0
    nit = tok // super_tok       # 1024-token iterations (16)
    gr = 2 * super_tok           # 2048-token granules (input and output)
    ngr = tok // gr
    half_ps = super_tok // 2     # 512 moving tokens per matmul

    nc = bacc.Bacc(
        "TRN2", target_bir_lowering=False, debug=False, num_devices=num_devices
    )
    # rows 0-127: e^T (d'), rows 128-255: o^T
    xt_d = nc.dram_tensor("xt", [D, tok], BF16, kind="ExternalInput").ap()
    # [d', {Ce^T | Co^T}] packed: ct[d', c*128 + k'] = C[2k'+c, d']
    ct_d = nc.dram_tensor("ct", [P, D], BF16, kind="ExternalInput").ap()
    # rows 0-127: even k' outputs, rows 128-255: odd
    out_d = nc.dram_tensor("out", [D, tok], BF16, kind="ExternalOutput").ap()

    with ExitStack() as ctx:
        tc = ctx.enter_context(tile.TileContext(nc))
        consts = ctx.enter_context(tc.tile_pool(name="consts", bufs=1))
        fill_pool = ctx.enter_context(tc.tile_pool(name="xfill", bufs=1))
        xin_pool = ctx.enter_context(tc.tile_pool(name="xin", bufs=5))
        out_sb_pool = ctx.enter_context(tc.tile_pool(name="out_sb", bufs=4))
        # Each PSUM tile spans 2 banks ([128, 1024] fp32); 4 bufs = all 8
        # banks, 2 iterations in flight.
        out_ps_pool = ctx.enter_context(
            tc.tile_pool(name="out_ps", bufs=4, space="PSUM")
        )

        # Stationary weights: 64KB, first on the scalar ring.
        ct_sb = consts.tile([P, 2, P], BF16)
        nc.scalar.dma_start(ct_sb[:], ct_d.rearrange("p (c k) -> p c k", k=P))

        x_q = xt_d.rearrange("(c p) (q t) -> q p c t", p=P, t=super_tok // 4)
        x_half = xt_d.rearrange("(c p) (h t) -> h p c t", p=P, t=super_tok // 2)
        x_fill = xt_d.rearrange("(c p) (i t) -> i p c t", p=P, t=super_tok)
        x_gr = xt_d.rearrange("(c p) (g t) -> g p c t", p=P, t=gr)

        xins = {}

        def stage_a_fill0():
            """Iteration 0 lands as 256/256/512-token tiles so the first
            matmuls start after only 128KB of input."""
            qa = fill_pool.tile([P, 2, super_tok // 4], BF16, name="xf0a")
            qb = fill_pool.tile([P, 2, super_tok // 4], BF16, name="xf0b")
            hc = fill_pool.tile([P, 2, super_tok // 2], BF16, name="xf0c")
            nc.sync.dma_start(qa[:], x_q[0])
            nc.sync.dma_start(qb[:], x_q[1])
            nc.sync.dma_start(hc[:], x_half[1])
            q = super_tok // 4
            # segments: (tile, tile_col0, iter_col0, width)
            xins[0] = [(qa, 0, 0, q), (qb, 0, q, q), (hc, 0, 2 * q, 2 * q)]

        def stage_a_fill1():
            xc = fill_pool.tile([P, 2, super_tok], BF16, name="xfill1")
            nc.gpsimd.dma_start(xc[:], x_fill[1])
            xins[1] = [(xc, 0, 0, super_tok)]

        # Granule rings: sync takes g2 (its fill work ends early), gpsimd
        # (SWDGE) the rest; sync then mostly ships odd-row outputs.
        GR_SYNC = {2}

        def stage_a(g):
            """Granule g covers iterations 2g, 2g+1 (g >= 1)."""
            if not (1 <= g < ngr):
                return
            xg = xin_pool.tile([P, 2, gr], BF16)
            eng = nc.sync if g in GR_SYNC else nc.gpsimd
            eng.dma_start(xg[:], x_gr[g])
            xins[2 * g] = [(xg, 0, 0, super_tok)]
            xins[2 * g + 1] = [(xg, super_tok, 0, super_tok)]

        pss_by_iter = {}

        def stage_b(i):
            """Per parity: one 2-bank PSUM tile [k' 128, tok 1024], filled
            by single-shot matmuls (contraction 128 = one weight tile),
            moving chunks <= 512 so no chunk straddles a PSUM bank."""
            if not (0 <= i < nit):
                return
            segs = xins.pop(i)
            pss = []
            for par in range(2):
                ps = out_ps_pool.tile([P, super_tok], F32)
                pss.append(ps)
                for (t, tcol0, icol0, width) in segs:
                    for w0 in range(0, width, half_ps):
                        w = min(half_ps, width - w0)
                        nc.tensor.matmul(
                            ps[:, icol0 + w0:icol0 + w0 + w],
                            ct_sb[:, par, :],
                            t[:, par, tcol0 + w0:tcol0 + w0 + w],
                            start=True,
                            stop=True,
                        )
            pss_by_iter[i] = pss

        out_sbs = {}

        # Odd-row out DMAs ride sync from granule 1 on; granule 0's odd
        # rows go on scalar (sync is still landing its input then).
        def out_rings(g):
            ring_e = nc.scalar
            ring_o = nc.scalar if g == 0 else nc.sync
            return ring_e, ring_o

        def stage_c(i):
            """PSUM->SBUF bf16 copies (even->DVE, odd->ACT) into 2-iter
            staging tiles; ship [128, 2048] per parity per granule."""
            if not (0 <= i < nit):
                return
            pss = pss_by_iter.pop(i)
            g, h = divmod(i, 2)
            if h == 0:
                sbe = out_sb_pool.tile([P, gr], BF16, name="sbe")
                sbo = out_sb_pool.tile([P, gr], BF16, name="sbo")
                out_sbs[g] = (sbe, sbo)
            sbe, sbo = out_sbs[g]
            sl = slice(h * super_tok, (h + 1) * super_tok)
            nc.vector.tensor_copy(sbe[:, sl], pss[0][:])
            nc.scalar.copy(sbo[:, sl], pss[1][:])
            cols = slice(g * gr, (g + 1) * gr)
            ring_e, ring_o = out_rings(g)
            if i >= nit - 2:
                # Tail: ship each iteration's halves immediately.
                icols = slice(i * super_tok, (i + 1) * super_tok)
                ring_e.dma_start(out_d[0:P, icols], sbe[:, sl])
                ring_o.dma_start(out_d[P:D, icols], sbo[:, sl])
                if h == 1:
                    out_sbs.pop(g)
            elif h == 1:
                ring_e.dma_start(out_d[0:P, cols], sbe[:])
                ring_o.dma_start(out_d[P:D, cols], sbo[:])
                out_sbs.pop(g)

        stage_a_fill0()
        stage_a_fill1()
        stage_a(1)
        for i in range(nit + 1):
            if i % 2 == 0:
                stage_a(i // 2 + 2)
            stage_b(i)
            stage_c(i - 1)

    nc.compile()
    return nc


_PROGRAM_CACHE: dict = {}


def _get_program() -> bass.Bass:
    if "nc" not in _PROGRAM_CACHE:
        _PROGRAM_CACHE["nc"] = build_program()
    return _PROGRAM_CACHE["nc"]


def make_in_maps(x_flat: np.ndarray) -> list[dict]:
    """x_flat: [B*N, D] float32. Per core upload xt = [e^T; o^T] bf16 and
    the packed stationary weights ct[d', c*128+k'] = C[2k'+c, d']."""
    C = dct_matrix().astype(np.float64)
    ce = C[0::2, 0:P].T  # [d', k'] even
    co = C[1::2, 0:P].T  # [d', k'] odd
    ct = np.concatenate([ce, co], axis=1).astype(NP_BF16)  # [128, 256]
    ct = np.ascontiguousarray(ct)

    xs = x_flat.reshape(N_CORES, TOK_PER_CORE, D)
    a = xs[:, :, 0:P].astype(np.float32)
    b = xs[:, :, :P - 1:-1].astype(np.float32)  # cols 255..128 = x[255-d']
    e = (a + b).astype(NP_BF16)
    o = (a - b).astype(NP_BF16)
    # [core, t, d'] -> [core, d', t]; stack e over o -> [core, 256, T]
    xt = np.concatenate(
        [e.transpose(0, 2, 1), o.transpose(0, 2, 1)], axis=1
    )
    xt = np.ascontiguousarray(xt)
    return [{"xt": xt[i], "ct": ct} for i in range(N_CORES)]


def kernel(x: np.ndarray) -> np.ndarray:
    x = np.ascontiguousarray(np.asarray(x, dtype=np.float32))
    b, n, d = x.shape
    assert (b, n, d) == (B, N, D), f"unexpected shape {x.shape}"
    nc = _get_program()
    in_maps = make_in_maps(x.reshape(b * n, d))
    res = run_bass_kernel_spmd(nc, in_maps, core_ids=list(range(N_CORES)))
    outs = []
    for r in res.results:
        od = np.asarray(r["out"]).astype(np.float32)  # [256, T] k-packed
        # out[t, 2k'+c] = od[c*128+k', t]
        outs.append(od.reshape(2, P, TOK_PER_CORE).transpose(2, 1, 0)
                    .reshape(TOK_PER_CORE, D))
    return np.concatenate(outs, axis=0).reshape(b, n, d)
